# revision 29
# baseline (speedup 1.0000x reference)
"""Trainium2 Bass kernel for nn_ExactAttention (block-diagonal sparse attention).

Reference computes dense softmax attention over [N,N] then masks to
block-diagonal segments (batch_seg is sorted).  Only the diagonal blocks
survive, so we compute segment-local attention only.

The reference subtracts the *global* max of Q@K^T before exp; softmax is
shift-invariant except through EPS=1e-8, whose effect is ~1e-8 relative
(denominators are O(100+)), far below fp32 noise, so we skip the max
entirely (max |dot| ~ 70 -> exp(70/sqrt(128)) ~ 450, no overflow).

Sharding: segments are sorted by length (desc) and dealt round-robin:
slot j of every core gets one of ranks [8j, 8j+8), all padded to the
group max L_j, so all 8 cores run one SPMD program with near-zero
padding waste and balanced work.

Precision/perf choices:
  * scores via bf16 hi/lo splitting (host-side):  K.Q^T = Kh.Qh + Kh.Ql
    + Kl.Qh, three full-rate (1 cycle/row) bf16 matmuls accumulated in
    fp32 PSUM.  Error ~2^-17 relative on the dot product (the dropped
    Kl.Ql term), below fp32 matmul noise.  4/3x faster than the PE's
    native fp32 mode (two half-rate passes).
  * AV in native fp32, V-stationary (one weight per key chunk):
    O^T [128 x m] += V_c^T P_c — keeps weight loads minimal and output
    in a layout the host can cheaply transpose.
  * denominator: S = sum_c P_c on DVE; host sums the 128 partitions.
    Padded key rows (zero K) give exp(0)=1; the host subtracts (L-len).
  * PE HAM warm-up: junk bf16 matmuls bridge the DMA wait so the clock
    throttle releases before real matmuls arrive.
"""

import numpy as np
import ml_dtypes

import concourse.bass as bass
import concourse.mybir as mybir
import concourse.tile as tile
from concourse import bacc
from concourse import bass_utils

D = 128
N_CORES = 8
EPS = 1e-8
F32 = mybir.dt.float32
BF16 = mybir.dt.bfloat16
BF16_NP = ml_dtypes.bfloat16

_program_cache = {}


def _build_program(slot_lens):
    """Build + compile the SPMD program for per-slot padded lengths."""
    key = tuple(slot_lens)
    if key in _program_cache:
        return _program_cache[key]

    scale = float(1.0 / np.sqrt(np.float32(D)))
    R = sum(slot_lens)
    offs = np.concatenate([[0], np.cumsum(slot_lens)]).astype(int)
    nkcs = [(L + 127) // 128 for L in slot_lens]
    choffs = np.concatenate([[0], np.cumsum(nkcs)]).astype(int)
    C = int(choffs[-1])
    max_nkc = max(nkcs)

    nc = bacc.Bacc("TRN2", target_bir_lowering=False, debug=False,
                   num_devices=N_CORES)

    # packed [qh | ql | kh | kl] per slot: big contiguous per-partition runs
    qk_d = nc.dram_tensor("qk", [D, 4 * R], BF16, kind="ExternalInput").ap()
    vx_d = nc.dram_tensor("vx", [D, C * 128], F32, kind="ExternalInput").ap()
    # merged [O^T | S] output: slot j, qblock qb0 at cols 2*offs[j]+2*qb0
    os_d = nc.dram_tensor("os", [D, 2 * R], F32, kind="ExternalOutput").ap()

    with tile.TileContext(nc) as tc:
        with tc.tile_pool(name="qk", bufs=2) as qk_pool, \
             tc.tile_pool(name="v", bufs=2) as v_pool, \
             tc.tile_pool(name="p", bufs=2 * max_nkc) as p_pool, \
             tc.tile_pool(name="osb", bufs=2) as o_pool, \
             tc.tile_pool(name="tps", bufs=3, space="PSUM") as t_psum, \
             tc.tile_pool(name="ops", bufs=2, space="PSUM") as o_psum:

            # PE warm-up: HAM releases the clock throttle only after ~3.4us
            # of sustained PE activity; junk bf16 matmuls bridge the initial
            # DMA wait so real matmuls start at 2.4GHz.
            with tc.tile_pool(name="warm", bufs=1) as warm_pool, \
                 tc.tile_pool(name="warmps", bufs=1, space="PSUM") as warm_psum:
                wsb = warm_pool.tile([128, 128], BF16)
                nc.vector.memset(wsb[:], 0.0)
                wps = warm_psum.tile([128, 128], F32)
                for _ in range(20):
                    nc.tensor.matmul(wps[:], wsb[:, :128], wsb[:],
                                     start=True, stop=True)

            for s, L in enumerate(slot_lens):
                nkc = nkcs[s]
                o0 = int(offs[s])
                c0 = int(choffs[s])
                qk_sb = qk_pool.tile([D, 4 * L], BF16, tag="qk")
                vs = v_pool.tile([D, nkc * 128], F32, tag="v")
                # big-burst DMA for all four Q/K pieces, split across both
                # HWDGE queues (sync+scalar) for 2x queue bandwidth; V on the
                # gpsimd SWDGE queue for extra DMA-queue parallelism
                nc.sync.dma_start(qk_sb[:, :2 * L], qk_d[:, 4 * o0:4 * o0 + 2 * L])
                nc.scalar.dma_start(qk_sb[:, 2 * L:],
                                    qk_d[:, 4 * o0 + 2 * L:4 * (o0 + L)])
                nc.gpsimd.dma_start(vs[:], vx_d[:, c0 * 128:(c0 + nkc) * 128])
                qhs = qk_sb[:, 0:L]
                qls = qk_sb[:, L:2 * L]
                khs = qk_sb[:, 2 * L:3 * L]
                kls = qk_sb[:, 3 * L:4 * L]

                # query blocks of <=512 (PSUM bank limit / moving-max)
                for qb0 in range(0, L, 512):
                    qbs = min(512, L - qb0)
                    p_tiles = []
                    for c in range(nkc):
                        ck = min(128, L - c * 128)
                        kslc = slice(c * 128, c * 128 + ck)
                        qslc = slice(qb0, qb0 + qbs)
                        t_ps = t_psum.tile([128, qbs], F32, tag="t")
                        nc.tensor.matmul(t_ps[:ck, :], khs[:, kslc],
                                         qhs[:, qslc], start=True, stop=False)
                        nc.tensor.matmul(t_ps[:ck, :], khs[:, kslc],
                                         qls[:, qslc], start=False, stop=False)
                        nc.tensor.matmul(t_ps[:ck, :], kls[:, kslc],
                                         qhs[:, qslc], start=False, stop=True)
                        p_sb = p_pool.tile([128, qbs], F32, tag="p")
                        nc.scalar.activation(p_sb[:ck, :], t_ps[:ck, :],
                                             mybir.ActivationFunctionType.Exp,
                                             scale=scale)
                        p_tiles.append(p_sb)

                    # AV: O^T += V_c^T P_c  (V stationary, one weight/chunk)
                    o_ps = o_psum.tile([128, qbs], F32, tag="ops")
                    for c in range(nkc):
                        ck = min(128, L - c * 128)
                        nc.tensor.matmul(o_ps[:],
                                         vs[:ck, c * 128:(c + 1) * 128],
                                         p_tiles[c][:ck, :],
                                         start=(c == 0), stop=(c == nkc - 1))

                    # Merged [O^T | S] tile.  S = sum_c P_c (DVE); host sums
                    # partitions for den.  Only the valid [:ck] partitions of
                    # each P tile are written by exp; partial chunks are
                    # slice-added so stale partitions never leak into S.
                    os_sb = o_pool.tile([128, 2 * qbs], F32, tag="o")
                    s_ap = os_sb[:, qbs:2 * qbs]
                    sck0 = min(128, L)
                    if sck0 < 128:
                        nc.gpsimd.memset(s_ap, 0.0)
                        nc.vector.tensor_add(s_ap[:sck0, :], s_ap[:sck0, :],
                                             p_tiles[0][:sck0, :])
                    else:
                        nc.vector.tensor_copy(s_ap, p_tiles[0][:])
                    for c in range(1, nkc):
                        ck = min(128, L - c * 128)
                        nc.vector.tensor_add(s_ap[:ck, :], s_ap[:ck, :],
                                             p_tiles[c][:ck, :])
                    d0 = 2 * o0 + 2 * qb0
                    # S can ship as soon as the adds finish (often before AV)
                    nc.sync.dma_start(os_d[:, d0 + qbs:d0 + 2 * qbs], s_ap)
                    # O^T copy+store split across engine pairs (DVE+sync,
                    # ACT+scalar) so the tail chains run in parallel
                    h = qbs // 2
                    nc.vector.tensor_copy(os_sb[:, :h], o_ps[:, :h])
                    nc.sync.dma_start(os_d[:, d0:d0 + h], os_sb[:, :h])
                    nc.scalar.copy(os_sb[:, h:qbs], o_ps[:, h:])
                    nc.scalar.dma_start(os_d[:, d0 + h:d0 + qbs],
                                        os_sb[:, h:qbs])

    nc.compile()
    _program_cache[key] = nc
    return nc


def _reference_host(Q, K, V, num_batch, batch_seg):
    """Pure-NumPy fallback for input shapes outside the tuned envelope."""
    dot = Q.astype(np.float64) @ K.T.astype(np.float64)
    A = np.exp((dot - dot.max()) / np.sqrt(np.float64(Q.shape[-1])))
    if num_batch > 1:
        A = np.where(batch_seg[None, :] == batch_seg[:, None], A, 0.0)
    return ((A / (A.sum(-1, keepdims=True) + EPS)) @ V.astype(np.float64)
            ).astype(np.float32)


def kernel(Q, K, V, num_batch, batch_seg):
    Q = np.asarray(Q, dtype=np.float32)
    K = np.asarray(K, dtype=np.float32)
    V = np.asarray(V, dtype=np.float32)
    batch_seg = np.asarray(batch_seg)
    N = Q.shape[0]
    nb = int(num_batch)

    counts = np.bincount(batch_seg.astype(np.int64), minlength=max(nb, 1))
    if nb < 2 or (counts.max() if nb else N) > 2048:
        return _reference_host(Q, K, V, nb, batch_seg)

    # row indices per segment (robust to unsorted batch_seg)
    row_order = np.argsort(batch_seg, kind="stable")
    starts = np.zeros(nb + 1, dtype=np.int64)
    np.cumsum(counts, out=starts[1:])

    # rank segments by length desc, group into slots of 8, then order slots
    # smallest-first: the first slot runs while the PE clock is still cold
    # and its load gates the pipeline start, so make it the cheapest.
    order = np.argsort(-counts, kind="stable")
    n_slots = (nb + N_CORES - 1) // N_CORES
    slot_lens = []
    assign = {}  # (core, slot) -> seg id
    for j in range(n_slots):
        grp = order[(n_slots - 1 - j) * N_CORES:(n_slots - j) * N_CORES]
        slot_lens.append(max(1, int(counts[grp].max())))
        for c, seg in enumerate(grp):
            assign[(c, j)] = int(seg)

    offs = np.concatenate([[0], np.cumsum(slot_lens)]).astype(int)
    nkcs = [(L + 127) // 128 for L in slot_lens]
    choffs = np.concatenate([[0], np.cumsum(nkcs)]).astype(int)
    R = int(offs[-1])
    C = int(choffs[-1])

    nc = _build_program(tuple(slot_lens))

    in_maps = []
    for core in range(N_CORES):
        Qp = np.zeros((R, D), np.float32)
        Kp = np.zeros((R, D), np.float32)
        Vp = np.zeros((C * 128, D), np.float32)
        for j in range(n_slots):
            seg = assign.get((core, j))
            if seg is None:
                continue
            b0, b1 = starts[seg], starts[seg + 1]
            ln = int(b1 - b0)
            if ln == 0:
                continue
            ridx = row_order[b0:b1]
            o0 = int(offs[j])
            Qp[o0:o0 + ln] = Q[ridx]
            Kp[o0:o0 + ln] = K[ridx]
            v0 = int(choffs[j]) * 128
            Vp[v0:v0 + ln] = V[ridx]
        qt = np.ascontiguousarray(Qp.T)
        kt = np.ascontiguousarray(Kp.T)
        qh = qt.astype(BF16_NP)
        ql = (qt - qh.astype(np.float32)).astype(BF16_NP)
        kh = kt.astype(BF16_NP)
        kl = (kt - kh.astype(np.float32)).astype(BF16_NP)
        qk = np.empty((D, 4 * R), BF16_NP)
        for j in range(n_slots):
            o0, L = int(offs[j]), slot_lens[j]
            qk[:, 4 * o0:4 * o0 + L] = qh[:, o0:o0 + L]
            qk[:, 4 * o0 + L:4 * o0 + 2 * L] = ql[:, o0:o0 + L]
            qk[:, 4 * o0 + 2 * L:4 * o0 + 3 * L] = kh[:, o0:o0 + L]
            qk[:, 4 * o0 + 3 * L:4 * o0 + 4 * L] = kl[:, o0:o0 + L]
        vh = np.ascontiguousarray(
            Vp.reshape(C, 128, D).transpose(1, 0, 2)).reshape(D, C * 128)
        in_maps.append({
            "qk": qk, "vx": vh,
        })

    global _last_in_maps
    _last_in_maps = in_maps
    res = bass_utils.run_bass_kernel_spmd(nc, in_maps,
                                          core_ids=list(range(N_CORES)))

    out = np.empty((N, D), np.float32)
    for (core, j), seg in assign.items():
        b0, b1 = starts[seg], starts[seg + 1]
        ln = int(b1 - b0)
        if ln == 0:
            continue
        o0 = int(offs[j])
        L = slot_lens[j]
        osr = res.results[core]["os"]                       # [D, 2R]
        # unpack per-qblock [ot(qbs) | s(qbs)] layout
        otT = np.empty((D, L), np.float32)
        sS = np.empty((D, L), np.float32)
        for qb0 in range(0, L, 512):
            qbs = min(512, L - qb0)
            d0 = 2 * o0 + 2 * qb0
            otT[:, qb0:qb0 + qbs] = osr[:, d0:d0 + qbs]
            sS[:, qb0:qb0 + qbs] = osr[:, d0 + qbs:d0 + 2 * qbs]
        # padded keys contribute exp(0)=1 each to the raw column sums
        den = sS[:, :ln].sum(axis=0, dtype=np.float64) - float(L - ln) + EPS
        out[row_order[b0:b1]] = (otT[:, :ln].T / den[:, None]).astype(np.float32)
    return out


# revision 31
# speedup vs baseline: 1.0131x; 1.0131x over previous
"""Trainium2 Bass kernel for nn_ExactAttention (block-diagonal sparse attention).

Reference computes dense softmax attention over [N,N] then masks to
block-diagonal segments (batch_seg is sorted).  Only the diagonal blocks
survive, so we compute segment-local attention only.

The reference subtracts the *global* max of Q@K^T before exp; softmax is
shift-invariant except through EPS=1e-8, whose effect is ~1e-8 relative
(denominators are O(100+)), far below fp32 noise, so we skip the max
entirely (max |dot| ~ 70 -> exp(70/sqrt(128)) ~ 450, no overflow).

Sharding: segments are sorted by length (desc) and dealt round-robin:
slot j of every core gets one of ranks [8j, 8j+8), all padded to the
group max L_j, so all 8 cores run one SPMD program with near-zero
padding waste and balanced work.

Precision/perf choices:
  * scores via bf16 hi/lo splitting (host-side):  K.Q^T = Kh.Qh + Kh.Ql
    + Kl.Qh, three full-rate (1 cycle/row) bf16 matmuls accumulated in
    fp32 PSUM.  Error ~2^-17 relative on the dot product (the dropped
    Kl.Ql term), below fp32 matmul noise.  4/3x faster than the PE's
    native fp32 mode (two half-rate passes).
  * AV in native fp32, V-stationary (one weight per key chunk):
    O^T [128 x m] += V_c^T P_c — keeps weight loads minimal and output
    in a layout the host can cheaply transpose.
  * denominator: S = sum_c P_c on DVE; host sums the 128 partitions.
    Padded key rows (zero K) give exp(0)=1; the host subtracts (L-len).
  * PE HAM warm-up: junk bf16 matmuls bridge the DMA wait so the clock
    throttle releases before real matmuls arrive.
"""

import numpy as np
import ml_dtypes

import concourse.bass as bass
import concourse.mybir as mybir
import concourse.tile as tile
from concourse import bacc
from concourse import bass_utils

D = 128
N_CORES = 8
EPS = 1e-8
F32 = mybir.dt.float32
BF16 = mybir.dt.bfloat16
BF16_NP = ml_dtypes.bfloat16

_program_cache = {}


def _build_program(slot_lens):
    """Build + compile the SPMD program for per-slot padded lengths."""
    key = tuple(slot_lens)
    if key in _program_cache:
        return _program_cache[key]

    scale = float(1.0 / np.sqrt(np.float32(D)))
    R = sum(slot_lens)
    offs = np.concatenate([[0], np.cumsum(slot_lens)]).astype(int)
    nkcs = [(L + 127) // 128 for L in slot_lens]
    choffs = np.concatenate([[0], np.cumsum(nkcs)]).astype(int)
    C = int(choffs[-1])
    max_nkc = max(nkcs)

    nc = bacc.Bacc("TRN2", target_bir_lowering=False, debug=False,
                   num_devices=N_CORES)

    # packed [qh | ql | kh | kl] per slot: big contiguous per-partition runs
    qk_d = nc.dram_tensor("qk", [D, 4 * R], BF16, kind="ExternalInput").ap()
    vx_d = nc.dram_tensor("vx", [D, C * 128], F32, kind="ExternalInput").ap()
    # merged [O^T | S] output: slot j, qblock qb0 at cols 2*offs[j]+2*qb0
    os_d = nc.dram_tensor("os", [D, 2 * R], F32, kind="ExternalOutput").ap()

    with tile.TileContext(nc) as tc:
        with tc.tile_pool(name="qk", bufs=2) as qk_pool, \
             tc.tile_pool(name="v", bufs=2) as v_pool, \
             tc.tile_pool(name="p", bufs=2 * max_nkc) as p_pool, \
             tc.tile_pool(name="osb", bufs=3) as o_pool, \
             tc.tile_pool(name="tps", bufs=4, space="PSUM") as t_psum, \
             tc.tile_pool(name="ops", bufs=2, space="PSUM") as o_psum:

            # PE warm-up: HAM releases the clock throttle only after ~3.4us
            # of sustained PE activity; junk bf16 matmuls bridge the initial
            # DMA wait so real matmuls start at 2.4GHz.
            with tc.tile_pool(name="warm", bufs=1) as warm_pool, \
                 tc.tile_pool(name="warmps", bufs=1, space="PSUM") as warm_psum:
                wsb = warm_pool.tile([128, 128], BF16)
                nc.vector.memset(wsb[:], 0.0)
                wps = warm_psum.tile([128, 128], F32)
                for _ in range(24):
                    nc.tensor.matmul(wps[:], wsb[:, :128], wsb[:],
                                     start=True, stop=True)

            for s, L in enumerate(slot_lens):
                nkc = nkcs[s]
                o0 = int(offs[s])
                c0 = int(choffs[s])
                qk_sb = qk_pool.tile([D, 4 * L], BF16, tag="qk")
                vs = v_pool.tile([D, nkc * 128], F32, tag="v")
                # big-burst DMA for all four Q/K pieces, split across both
                # HWDGE queues (sync+scalar) for 2x queue bandwidth; V on the
                # gpsimd SWDGE queue for extra DMA-queue parallelism
                nc.sync.dma_start(qk_sb[:, :2 * L], qk_d[:, 4 * o0:4 * o0 + 2 * L])
                nc.scalar.dma_start(qk_sb[:, 2 * L:],
                                    qk_d[:, 4 * o0 + 2 * L:4 * (o0 + L)])
                nc.gpsimd.dma_start(vs[:], vx_d[:, c0 * 128:(c0 + nkc) * 128])
                qhs = qk_sb[:, 0:L]
                qls = qk_sb[:, L:2 * L]
                khs = qk_sb[:, 2 * L:3 * L]
                kls = qk_sb[:, 3 * L:4 * L]

                # query blocks of <=512 (PSUM bank limit / moving-max)
                for qb0 in range(0, L, 512):
                    qbs = min(512, L - qb0)
                    p_tiles = []
                    for c in range(nkc):
                        ck = min(128, L - c * 128)
                        kslc = slice(c * 128, c * 128 + ck)
                        qslc = slice(qb0, qb0 + qbs)
                        t_ps = t_psum.tile([128, qbs], F32, tag="t")
                        nc.tensor.matmul(t_ps[:ck, :], khs[:, kslc],
                                         qhs[:, qslc], start=True, stop=False)
                        nc.tensor.matmul(t_ps[:ck, :], khs[:, kslc],
                                         qls[:, qslc], start=False, stop=False)
                        nc.tensor.matmul(t_ps[:ck, :], kls[:, kslc],
                                         qhs[:, qslc], start=False, stop=True)
                        p_sb = p_pool.tile([128, qbs], F32, tag="p")
                        nc.scalar.activation(p_sb[:ck, :], t_ps[:ck, :],
                                             mybir.ActivationFunctionType.Exp,
                                             scale=scale)
                        p_tiles.append(p_sb)

                    # AV: O^T += V_c^T P_c  (V stationary, one weight/chunk)
                    o_ps = o_psum.tile([128, qbs], F32, tag="ops")
                    for c in range(nkc):
                        ck = min(128, L - c * 128)
                        nc.tensor.matmul(o_ps[:],
                                         vs[:ck, c * 128:(c + 1) * 128],
                                         p_tiles[c][:ck, :],
                                         start=(c == 0), stop=(c == nkc - 1))

                    # Merged [O^T | S] tile.  S = sum_c P_c (DVE); host sums
                    # partitions for den.  Only the valid [:ck] partitions of
                    # each P tile are written by exp; partial chunks are
                    # slice-added so stale partitions never leak into S.
                    os_sb = o_pool.tile([128, 2 * qbs], F32, tag="o")
                    s_ap = os_sb[:, qbs:2 * qbs]
                    sck0 = min(128, L)
                    if sck0 < 128:
                        nc.gpsimd.memset(s_ap, 0.0)
                        nc.vector.tensor_add(s_ap[:sck0, :], s_ap[:sck0, :],
                                             p_tiles[0][:sck0, :])
                    else:
                        nc.vector.tensor_copy(s_ap, p_tiles[0][:])
                    for c in range(1, nkc):
                        ck = min(128, L - c * 128)
                        nc.vector.tensor_add(s_ap[:ck, :], s_ap[:ck, :],
                                             p_tiles[c][:ck, :])
                    d0 = 2 * o0 + 2 * qb0
                    # S can ship as soon as the adds finish (often before AV)
                    nc.sync.dma_start(os_d[:, d0 + qbs:d0 + 2 * qbs], s_ap)
                    # O^T copy+store split across engine pairs (DVE+sync,
                    # ACT+scalar) so the tail chains run in parallel
                    h = qbs // 2
                    nc.vector.tensor_copy(os_sb[:, :h], o_ps[:, :h])
                    nc.sync.dma_start(os_d[:, d0:d0 + h], os_sb[:, :h])
                    nc.scalar.copy(os_sb[:, h:qbs], o_ps[:, h:])
                    nc.scalar.dma_start(os_d[:, d0 + h:d0 + qbs],
                                        os_sb[:, h:qbs])

    nc.compile()
    _program_cache[key] = nc
    return nc


def _reference_host(Q, K, V, num_batch, batch_seg):
    """Pure-NumPy fallback for input shapes outside the tuned envelope."""
    dot = Q.astype(np.float64) @ K.T.astype(np.float64)
    A = np.exp((dot - dot.max()) / np.sqrt(np.float64(Q.shape[-1])))
    if num_batch > 1:
        A = np.where(batch_seg[None, :] == batch_seg[:, None], A, 0.0)
    return ((A / (A.sum(-1, keepdims=True) + EPS)) @ V.astype(np.float64)
            ).astype(np.float32)


def kernel(Q, K, V, num_batch, batch_seg):
    Q = np.asarray(Q, dtype=np.float32)
    K = np.asarray(K, dtype=np.float32)
    V = np.asarray(V, dtype=np.float32)
    batch_seg = np.asarray(batch_seg)
    N = Q.shape[0]
    nb = int(num_batch)

    counts = np.bincount(batch_seg.astype(np.int64), minlength=max(nb, 1))
    if nb < 2 or (counts.max() if nb else N) > 2048:
        return _reference_host(Q, K, V, nb, batch_seg)

    # row indices per segment (robust to unsorted batch_seg)
    row_order = np.argsort(batch_seg, kind="stable")
    starts = np.zeros(nb + 1, dtype=np.int64)
    np.cumsum(counts, out=starts[1:])

    # rank segments by length desc, group into slots of 8, then order slots
    # smallest-first: the first slot runs while the PE clock is still cold
    # and its load gates the pipeline start, so make it the cheapest.
    order = np.argsort(-counts, kind="stable")
    n_slots = (nb + N_CORES - 1) // N_CORES
    slot_lens = []
    assign = {}  # (core, slot) -> seg id
    for j in range(n_slots):
        grp = order[(n_slots - 1 - j) * N_CORES:(n_slots - j) * N_CORES]
        slot_lens.append(max(1, int(counts[grp].max())))
        for c, seg in enumerate(grp):
            assign[(c, j)] = int(seg)

    offs = np.concatenate([[0], np.cumsum(slot_lens)]).astype(int)
    nkcs = [(L + 127) // 128 for L in slot_lens]
    choffs = np.concatenate([[0], np.cumsum(nkcs)]).astype(int)
    R = int(offs[-1])
    C = int(choffs[-1])

    nc = _build_program(tuple(slot_lens))

    in_maps = []
    for core in range(N_CORES):
        Qp = np.zeros((R, D), np.float32)
        Kp = np.zeros((R, D), np.float32)
        Vp = np.zeros((C * 128, D), np.float32)
        for j in range(n_slots):
            seg = assign.get((core, j))
            if seg is None:
                continue
            b0, b1 = starts[seg], starts[seg + 1]
            ln = int(b1 - b0)
            if ln == 0:
                continue
            ridx = row_order[b0:b1]
            o0 = int(offs[j])
            Qp[o0:o0 + ln] = Q[ridx]
            Kp[o0:o0 + ln] = K[ridx]
            v0 = int(choffs[j]) * 128
            Vp[v0:v0 + ln] = V[ridx]
        qt = np.ascontiguousarray(Qp.T)
        kt = np.ascontiguousarray(Kp.T)
        qh = qt.astype(BF16_NP)
        ql = (qt - qh.astype(np.float32)).astype(BF16_NP)
        kh = kt.astype(BF16_NP)
        kl = (kt - kh.astype(np.float32)).astype(BF16_NP)
        qk = np.empty((D, 4 * R), BF16_NP)
        for j in range(n_slots):
            o0, L = int(offs[j]), slot_lens[j]
            qk[:, 4 * o0:4 * o0 + L] = qh[:, o0:o0 + L]
            qk[:, 4 * o0 + L:4 * o0 + 2 * L] = ql[:, o0:o0 + L]
            qk[:, 4 * o0 + 2 * L:4 * o0 + 3 * L] = kh[:, o0:o0 + L]
            qk[:, 4 * o0 + 3 * L:4 * o0 + 4 * L] = kl[:, o0:o0 + L]
        vh = np.ascontiguousarray(
            Vp.reshape(C, 128, D).transpose(1, 0, 2)).reshape(D, C * 128)
        in_maps.append({
            "qk": qk, "vx": vh,
        })

    global _last_in_maps
    _last_in_maps = in_maps
    res = bass_utils.run_bass_kernel_spmd(nc, in_maps,
                                          core_ids=list(range(N_CORES)))

    out = np.empty((N, D), np.float32)
    for (core, j), seg in assign.items():
        b0, b1 = starts[seg], starts[seg + 1]
        ln = int(b1 - b0)
        if ln == 0:
            continue
        o0 = int(offs[j])
        L = slot_lens[j]
        osr = res.results[core]["os"]                       # [D, 2R]
        # unpack per-qblock [ot(qbs) | s(qbs)] layout
        otT = np.empty((D, L), np.float32)
        sS = np.empty((D, L), np.float32)
        for qb0 in range(0, L, 512):
            qbs = min(512, L - qb0)
            d0 = 2 * o0 + 2 * qb0
            otT[:, qb0:qb0 + qbs] = osr[:, d0:d0 + qbs]
            sS[:, qb0:qb0 + qbs] = osr[:, d0 + qbs:d0 + 2 * qbs]
        # padded keys contribute exp(0)=1 each to the raw column sums
        den = sS[:, :ln].sum(axis=0, dtype=np.float64) - float(L - ln) + EPS
        out[row_order[b0:b1]] = (otT[:, :ln].T / den[:, None]).astype(np.float32)
    return out
